# revision 33
# baseline (speedup 1.0000x reference)
"""Trainium2 Bass kernel for nn_ChannelDropout (topk channel masking).

Reference computation (per batch image b of x[B, C, H, W]):
    y    = mean(x[b], spatial) + max(x[b], spatial)          # [C]
    h    = prelu(y @ w1 + b1)                                # [C/16]
    y2   = sigmoid(h @ w2 + b2)                              # [C]
    thr  = k-th largest of y2 (k = C/2)
    mask = (y2 < thr)
    keep = where(rand[b] < 0.5, mask * y2, y2)               # [C]
    out[b] = x[b] * keep[:, None, None]

Strategy: pure data parallel over 8 NeuronCores (4 batch images per core).
Per core, x is processed as [128 channels, spatial] tiles:
  - spatial sum on ScalarE (activation Copy + accum_out); the Copy's main
    output doubles as the bf16 output staging tile
  - spatial max on GpSimd/VectorE (reduce_max)
  - tiny FC on TensorE (fp32 matmuls, bias via augmented contraction)
  - top-k mask via exact rank counting: for each channel c,
    count{c' : z[c'] > z[c]} >= k  <=>  y2[c] < thr  (ranking done on the
    pre-sigmoid logits z, which is equivalent and avoids LUT monotonicity
    concerns). The row-vs-column copies of z are produced by fp32 matmuls
    against 1.0 which are bit-exact, so comparisons are self-consistent.
  - final per-channel scale applied in-place on the bf16 staging tile,
    which is streamed back to HBM as bf16 (host upcasts to f32; the
    kernel-side rounding is ~2^-9 relative, far inside tolerance)
"""

import numpy as np

import concourse.bacc as bacc
import concourse.mybir as mybir
from concourse import tile
from concourse.bass_utils import run_bass_kernel_spmd

f32 = mybir.dt.float32
bf16 = mybir.dt.bfloat16
Alu = mybir.AluOpType
Act = mybir.ActivationFunctionType
Ax = mybir.AxisListType

B, C, H, W = 32, 512, 56, 56
S = H * W                 # 3136
NCORES = 8
BP = B // NCORES          # 4 batches per core
HID = C // 16             # 32
KTOP = C // 2             # 256
P = 128
NBLK = C // P             # 4 channel blocks
COLS = BP * NBLK          # 16 (col = b*NBLK + m)

# =========================================================================
# v2 builder: bf16 output staging, max offloaded from DVE, split DMA rings
# =========================================================================

V2_OPTS = dict(
    xbufs=8,              # f32 x tile ring depth (tiles of [128, S])
    obufs=12,             # bf16 out staging ring depth (must cover the
                          # ~2-batch stats->store lifetime of a staging tile)
    max_engine="dve",     # "dve" | "gpsimd" | "tsmax_gp" | "tsmax_gp_ip"
                          # | "split_ts": spatial max engine
    mul_engine="dve",     # "dve" | "act" | "gpsimd" | "dve3act1" |
                          # "dve2act2": final scale engine(s)
    cmp_engine="act",     # "act" (Sign+bias from PSUM) | "dve" (is_gt)
    store_engine="scalar",  # "auto" | "sync" | "scalar" | "gpsimd"
                            # | "mixed" (scalar/gpsimd alternating by block)
    load_engine="sync",
    out_dtype="bf16",     # "bf16" | "f32"
    fuse_prelu=False,     # single stt reading h PSUM twice: ISA-invalid
                          # (two reads of one PSUM operand); keep False
    split_stt=True,       # per-block stats stt so the FC can start before
                          # the last block's max lands
    pipeline=True,        # emit batch b's mask chain after batch b+1's
                          # stats pass (1-deep software pipeline)
    zcol_direct=False,    # z columns via wb2-slice matmuls (shorter chain)
                          # instead of transposing the z row
    tail_flush=True,      # drain the pending chain before the last batch's
                          # stats so its stores fill the read-drained window
    tail_split=True,      # last batch only: split scale across DVE+ACT and
                          # stores across scalar+gpsimd queues (halves the
                          # exposed serial tail; steady state untouched)
    mode="full",          # "full" | "nomask" (skip stats/mask; time DMA+sum)
)


def _build_v2(a_val: float, reps: int = 1, **over):
    opts = dict(V2_OPTS, **over)
    mode = opts["mode"]
    odt = bf16 if opts["out_dtype"] == "bf16" else f32

    nc = bacc.Bacc("TRN2", target_bir_lowering=False, debug=False,
                   num_devices=NCORES)

    x_d = nc.dram_tensor("x", [BP, C, S], f32, kind="ExternalInput")
    rand_d = nc.dram_tensor("rand", [BP, C], f32, kind="ExternalInput")
    w1_d = nc.dram_tensor("w1", [C, HID], f32, kind="ExternalInput")
    b1_d = nc.dram_tensor("b1", [1, HID], f32, kind="ExternalInput")
    w2_d = nc.dram_tensor("w2", [HID, C], f32, kind="ExternalInput")
    b2_d = nc.dram_tensor("b2", [1, C], f32, kind="ExternalInput")
    out_d = nc.dram_tensor("out", [BP, C, S], odt, kind="ExternalOutput")

    prelu_op1 = Alu.max if a_val <= 1.0 else Alu.min

    with tile.TileContext(nc) as tc:
        with (
            tc.tile_pool(name="const", bufs=1) as const,
            tc.tile_pool(name="xp", bufs=opts["xbufs"]) as xp,
            tc.tile_pool(name="op", bufs=opts["obufs"]) as op,
            tc.tile_pool(name="trashp", bufs=2) as trashp,
            tc.tile_pool(name="rowp", bufs=2) as rowp,
            tc.tile_pool(name="bcp", bufs=2) as bcp,
            tc.tile_pool(name="cmpp", bufs=2) as cmpp,
            tc.tile_pool(name="smallp", bufs=2) as smallp,
            tc.tile_pool(name="ps_h", bufs=2, space="PSUM") as ps_h,
            tc.tile_pool(name="ps_z", bufs=2, space="PSUM") as ps_z,
            tc.tile_pool(name="ps_zb", bufs=2, space="PSUM") as ps_zb,
            tc.tile_pool(name="ps_t", bufs=2, space="PSUM") as ps_t,
        ):
            ld_eng = {"sync": nc.sync, "scalar": nc.scalar,
                      "vector": nc.vector, "gpsimd": nc.gpsimd
                      }[opts["load_engine"]]
            # stores issue on the engine that produced the scaled tile, so
            # the dma_start's dependency wait is trivially satisfied and
            # never head-of-line-blocks other work on that engine's queue
            st_name = opts["store_engine"]
            if st_name == "auto":
                st_name = "scalar"

            def st_eng_m(m):
                if st_name == "mixed":
                    return nc.scalar if m % 2 == 0 else nc.gpsimd
                return {"sync": nc.sync, "scalar": nc.scalar,
                        "gpsimd": nc.gpsimd}[st_name]

            def max_eng(m):
                return {"dve": nc.vector,
                        "gpsimd": nc.gpsimd}[opts["max_engine"]]

            def mul_on_act(m):
                me = opts["mul_engine"]
                return (me == "act" or (me == "dve3act1" and m == 3)
                        or (me == "dve2act2" and m >= 2))

            def mul_eng(m):
                return nc.gpsimd if opts["mul_engine"] == "gpsimd" \
                    else nc.vector

            # ---- constants (small DMAs on the ACT HWDGE ring) ----
            w1_sb = const.tile([P, NBLK, HID], f32)
            nc.scalar.dma_start(w1_sb[:], w1_d.ap().rearrange("(k p) j -> p k j", p=P))
            wb2_sb = const.tile([HID + 1, C], f32)
            nc.scalar.dma_start(wb2_sb[0:HID, :], w2_d.ap())
            nc.scalar.dma_start(wb2_sb[HID:HID + 1, :], b2_d.ap())
            b1_sb = const.tile([1, HID], f32)
            nc.scalar.dma_start(b1_sb[:], b1_d.ap())
            rand_rows = []
            for b in range(BP):
                rrow = const.tile([1, C], f32, name=f"rand_row{b}")
                nc.scalar.dma_start(rrow[:], rand_d.ap()[b:b + 1, :])
                rand_rows.append(rrow)
            ones128 = const.tile([1, P], f32)
            nc.vector.memset(ones128[:], 1.0)
            one1 = const.tile([1, 1], f32)
            nc.vector.memset(one1[:], 1.0)
            hT1 = const.tile([HID + 1, BP], f32)
            nc.vector.memset(hT1[HID:HID + 1, :], 1.0)

            # rand -> column layout + rb = (rand < 0.5), once for all reps
            rb_sb = const.tile([P, COLS], f32, name="rb_sb")
            for b in range(BP):
                rt_ps = ps_t.tile([P, NBLK], f32, name="rt_ps", tag="t")
                for m in range(NBLK):
                    nc.tensor.matmul(rt_ps[:, m:m + 1],
                                     rand_rows[b][:, m * P:(m + 1) * P],
                                     one1[:], start=True, stop=True)
                nc.vector.tensor_scalar(
                    rb_sb[:, b * NBLK:(b + 1) * NBLK], rt_ps[:], 0.5, None,
                    op0=Alu.is_lt)

            for rep in range(reps):
                sums = smallp.tile([P, COLS], f32, name="sums", tag="sums")
                maxs = smallp.tile([P, COLS], f32, name="maxs", tag="maxs")
                stats = smallp.tile([P, COLS], f32, name="stats", tag="stats")
                gts = smallp.tile([P, COLS], f32, name="gts", tag="gts")
                zcb = smallp.tile([P, COLS], f32, name="zcb", tag="zcb")
                y2cb = smallp.tile([P, COLS], f32, name="y2cb", tag="y2cb")
                mask_sb = smallp.tile([P, COLS], f32, name="mask_sb", tag="mask")
                u_sb = smallp.tile([P, COLS], f32, name="u_sb", tag="u")
                v_sb = smallp.tile([P, COLS], f32, name="v_sb", tag="v")
                fmap = smallp.tile([P, COLS], f32, name="fmap", tag="fmap")

                def emit_load_stats(b):
                    """Loads + stats pass for batch b; returns the bf16
                    staging tiles (x tiles are dead after this)."""
                    xts, ots = [], []
                    for m in range(NBLK):
                        xt = xp.tile([P, S], f32, name="xt", tag="xt")
                        ld_eng.dma_start(
                            xt[:], x_d.ap()[b, m * P:(m + 1) * P, :])
                        xts.append(xt)
                        ots.append(op.tile([P, S], odt, name="ot", tag="ot"))
                    for m in range(NBLK):
                        col = b * NBLK + m
                        nc.scalar.activation(ots[m][:], xts[m][:], Act.Copy,
                                             accum_out=sums[:, col:col + 1])
                        if mode == "nomask":
                            continue
                        me = opts["max_engine"]
                        if me in ("tsmax_gp", "tsmax_gp_ip") or (
                                me == "split_ts" and m % 2 == 1):
                            if me == "tsmax_gp_ip":
                                tr = xts[m]  # x dead after stats; in-place
                            else:
                                tr = trashp.tile([P, S], f32,
                                                 name="tr", tag="tr")
                            nc.gpsimd.tensor_scalar(
                                tr[:], xts[m][:], 1.0, None,
                                op0=Alu.mult, op1=Alu.max,
                                accum_out=maxs[:, col:col + 1])
                        else:
                            eng = nc.vector if me in ("dve", "split_ts") \
                                else max_eng(m)
                            eng.reduce_max(maxs[:, col:col + 1],
                                           xts[m][:], axis=Ax.X)
                        if opts["split_stt"]:
                            nc.vector.scalar_tensor_tensor(
                                stats[:, col:col + 1], sums[:, col:col + 1],
                                1.0 / S, maxs[:, col:col + 1],
                                op0=Alu.mult, op1=Alu.add)
                    if mode == "nomask":
                        for m in range(NBLK):
                            st_eng_m(m).dma_start(
                                out_d.ap()[b, m * P:(m + 1) * P, :],
                                ots[m][:])
                    return ots

                def emit_chain(b, ots):
                    """FC + topk mask + scale + store for batch b. Emitted
                    one batch behind the stats pass (software pipeline), so
                    neither ACT nor DVE ever queue-blocks the next batch's
                    stats behind this chain's cross-engine waits."""
                    sl = slice(b * NBLK, (b + 1) * NBLK)
                    if not opts["split_stt"]:
                        nc.vector.scalar_tensor_tensor(
                            stats[:, sl], sums[:, sl], 1.0 / S, maxs[:, sl],
                            op0=Alu.mult, op1=Alu.add)

                    # ---- FC: h = prelu(y @ w1 + b1) as hT column ----
                    h_ps = ps_h.tile([HID, 1], f32, name="h_ps", tag="h")
                    for k in range(NBLK):
                        ck = b * NBLK + k
                        nc.tensor.matmul(h_ps[:], w1_sb[:, k, :],
                                         stats[:, ck:ck + 1],
                                         start=(k == 0), stop=False)
                    nc.tensor.matmul(h_ps[:], b1_sb[:], one1[:],
                                     start=False, stop=True)
                    if opts["fuse_prelu"]:
                        nc.vector.scalar_tensor_tensor(
                            hT1[0:HID, b:b + 1], h_ps[:], a_val, h_ps[:],
                            op0=Alu.mult, op1=prelu_op1)
                    else:
                        h_sb = smallp.tile([HID, 1], f32, name="h_sb",
                                           tag="h_sb")
                        nc.vector.tensor_copy(h_sb[:], h_ps[:])
                        nc.vector.scalar_tensor_tensor(
                            hT1[0:HID, b:b + 1], h_sb[:], a_val, h_sb[:],
                            op0=Alu.mult, op1=prelu_op1)

                    # ---- z = hT1 @ [w2; b2]  -> logits row [1, C] ----
                    z_ps = ps_z.tile([1, C], f32, name="z_ps", tag="z")
                    nc.tensor.matmul(z_ps[:], hT1[:, b:b + 1], wb2_sb[:],
                                     start=True, stop=True)
                    z_sb = rowp.tile([1, C], f32, name="z_sb", tag="z_sb")
                    nc.vector.tensor_copy(z_sb[:], z_ps[:])

                    # broadcast logits row to all 128 partitions (bit-exact)
                    zb_ps = ps_zb.tile([P, C], f32, name="zb_ps", tag="zb")
                    nc.tensor.matmul(zb_ps[:], ones128[:], z_sb[:],
                                     start=True, stop=True)
                    if opts["cmp_engine"] == "dve":
                        zb_sb = bcp.tile([P, C], f32, name="zb_sb",
                                         tag="zb_sb")
                        nc.vector.tensor_copy(zb_sb[:], zb_ps[:])

                    # logits in column layout. Either transpose the z row
                    # (via matmuls against 1.0, bit-exact by construction) or
                    # recompute z per column block from wb2 directly (same
                    # contraction order on the PE array -> same bits; starts
                    # earlier since it doesn't wait for the z row copy).
                    # For the ACT Sign compare the column is stored NEGATED
                    # (exact) so it can serve as the per-partition bias.
                    t_ps = ps_t.tile([P, NBLK], f32, name="t_ps", tag="t")
                    for m in range(NBLK):
                        if opts["zcol_direct"]:
                            nc.tensor.matmul(t_ps[:, m:m + 1],
                                             wb2_sb[:, m * P:(m + 1) * P],
                                             hT1[:, b:b + 1],
                                             start=True, stop=True)
                        else:
                            nc.tensor.matmul(t_ps[:, m:m + 1],
                                             z_sb[:, m * P:(m + 1) * P],
                                             one1[:], start=True, stop=True)
                    if opts["cmp_engine"] == "act":
                        nc.vector.tensor_scalar(zcb[:, sl], t_ps[:], -1.0,
                                                None, op0=Alu.mult)
                    else:
                        nc.vector.tensor_copy(zcb[:, sl], t_ps[:])
                    # sigmoid only on the column copy (the values we use)
                    nc.scalar.activation(y2cb[:, sl], t_ps[:], Act.Sigmoid)

                    # ---- exact rank counts ----
                    # dve: gts[c] = #{c' : z[c'] > z[c]}; mask = gts >= k
                    # act: gts[c] = sum_c' sign(z[c'] - z[c]) = 2*#gt - (C-1)
                    #      (z generically distinct); mask = gts >= 1
                    for m in range(NBLK):
                        col = b * NBLK + m
                        if opts["cmp_engine"] == "act":
                            cmp_t = cmpp.tile([P, C], bf16, name="cmp_t",
                                              tag="cmp")
                            nc.scalar.activation(
                                cmp_t[:], zb_ps[:], Act.Sign,
                                bias=zcb[:, col:col + 1],
                                accum_out=gts[:, col:col + 1])
                        else:
                            cmp_t = cmpp.tile([P, C], f32, name="cmp_t",
                                              tag="cmp")
                            nc.vector.tensor_scalar(
                                cmp_t[:], zb_sb[:], zcb[:, col:col + 1], None,
                                op0=Alu.is_gt, op1=Alu.add,
                                accum_out=gts[:, col:col + 1])

                    # ---- final map: fmap = y2 * (mask*rb + (1 - rb)) ----
                    mask_thr = 1.0 if opts["cmp_engine"] == "act" \
                        else float(KTOP)
                    nc.vector.tensor_scalar(mask_sb[:, sl], gts[:, sl],
                                            mask_thr, None, op0=Alu.is_ge)
                    nc.vector.tensor_tensor(u_sb[:, sl], rb_sb[:, sl],
                                            mask_sb[:, sl], op=Alu.mult)
                    nc.vector.scalar_tensor_tensor(
                        v_sb[:, sl], rb_sb[:, sl], -1.0, u_sb[:, sl],
                        op0=Alu.mult, op1=Alu.add)
                    nc.vector.scalar_tensor_tensor(
                        fmap[:, sl], v_sb[:, sl], 1.0, y2cb[:, sl],
                        op0=Alu.add, op1=Alu.mult)

                    # ---- scale the bf16 staging tiles in place and store ----
                    tail = opts["tail_split"] and b == BP - 1
                    for m in range(NBLK):
                        col = b * NBLK + m
                        if mul_on_act(m) or (tail and m % 2 == 1):
                            nc.scalar.activation(ots[m][:], ots[m][:],
                                                 Act.Copy,
                                                 scale=fmap[:, col:col + 1])
                        else:
                            mul_eng(m).tensor_scalar(ots[m][:], ots[m][:],
                                                     fmap[:, col:col + 1],
                                                     None, op0=Alu.mult)
                        se = nc.gpsimd if (tail and m % 2 == 1) \
                            else st_eng_m(m)
                        se.dma_start(
                            out_d.ap()[b, m * P:(m + 1) * P, :], ots[m][:])

                pend = None
                for b in range(BP):
                    # tail flush: the last batch's sums are gated on the
                    # final loads anyway, so drain the pending chain first —
                    # its stores fill the otherwise-idle DMA window
                    if (pend is not None and b == BP - 1
                            and opts["tail_flush"]):
                        emit_chain(*pend)
                        pend = None
                    ots_b = emit_load_stats(b)
                    if mode == "nomask":
                        continue
                    if not opts["pipeline"]:
                        emit_chain(b, ots_b)
                        continue
                    if pend is not None:
                        emit_chain(*pend)
                        pend = None
                    pend = (b, ots_b)
                if pend is not None:
                    emit_chain(*pend)

    nc.compile()
    return nc


# =========================================================================
# v1 builder (previous session's f32 kernel, kept for A/B comparison)
# =========================================================================

DEFAULT_OPTS = dict(
    batch_tiles=True,    # legacy switch: False = tile_blocks 1
    tile_blocks=4,        # channel blocks per x tile (4=batch, 2=half, 1=block)
    store_engine="sync",  # "sync" | "scalar" | "gpsimd" ring for stores
    mode="full",          # "full" | "dmaonly" | "nostore"
    xbufs=None,           # x tile ring depth in chunks (default 12//tile_blocks)
    mul_engine="dve",     # "dve" | "act" | "split": engine for final scaling
    max_engine="dve",     # "dve" (reduce_max) | "tsmax" (tensor_scalar+accum)
    use_b1=False,
    use_b2=False,
    cmp_from_psum=True,
)


def _build(a_val: float, reps: int = 1, **over):
    opts = dict(DEFAULT_OPTS, **over)
    mode = opts["mode"]
    TB = opts["tile_blocks"] if opts["batch_tiles"] else 1
    xbufs = opts["xbufs"]
    if xbufs is None:
        xbufs = 12 // TB

    nc = bacc.Bacc("TRN2", target_bir_lowering=False, debug=False,
                   num_devices=NCORES)

    x_d = nc.dram_tensor("x", [BP, C, S], f32, kind="ExternalInput")
    rand_d = nc.dram_tensor("rand", [BP, C], f32, kind="ExternalInput")
    w1_d = nc.dram_tensor("w1", [C, HID], f32, kind="ExternalInput")
    b1_d = nc.dram_tensor("b1", [1, HID], f32, kind="ExternalInput")
    w2_d = nc.dram_tensor("w2", [HID, C], f32, kind="ExternalInput")
    b2_d = nc.dram_tensor("b2", [1, C], f32, kind="ExternalInput")
    out_d = nc.dram_tensor("out", [BP, C, S], f32, kind="ExternalOutput")

    prelu_op1 = Alu.max if a_val <= 1.0 else Alu.min

    with tile.TileContext(nc) as tc:
        with (
            tc.tile_pool(name="const", bufs=1) as const,
            tc.tile_pool(name="xp", bufs=xbufs) as xp,
            tc.tile_pool(name="trashp", bufs=2) as trashp,
            tc.tile_pool(name="rowp", bufs=2) as rowp,
            tc.tile_pool(name="bcp", bufs=2) as bcp,
            tc.tile_pool(name="cmpp", bufs=2) as cmpp,
            tc.tile_pool(name="smallp", bufs=2) as smallp,
            tc.tile_pool(name="ps_h", bufs=2, space="PSUM") as ps_h,
            tc.tile_pool(name="ps_z", bufs=2, space="PSUM") as ps_z,
            tc.tile_pool(name="ps_zb", bufs=2, space="PSUM") as ps_zb,
            tc.tile_pool(name="ps_t", bufs=2, space="PSUM") as ps_t,
        ):
            st_eng = {"sync": nc.sync, "scalar": nc.scalar,
                      "gpsimd": nc.gpsimd}[opts["store_engine"]]

            # ---- constants (small DMAs on the ACT HWDGE ring) ----
            w1_sb = const.tile([P, NBLK, HID], f32)
            nc.scalar.dma_start(w1_sb[:], w1_d.ap().rearrange("(k p) j -> p k j", p=P))
            wb2_sb = const.tile([HID + 1, C], f32)
            nc.scalar.dma_start(wb2_sb[0:HID, :], w2_d.ap())
            nc.scalar.dma_start(wb2_sb[HID:HID + 1, :], b2_d.ap())
            b1_sb = const.tile([1, HID], f32)
            nc.scalar.dma_start(b1_sb[:], b1_d.ap())
            rand_rows = []
            for b in range(BP):
                rrow = const.tile([1, C], f32, name=f"rand_row{b}")
                nc.scalar.dma_start(rrow[:], rand_d.ap()[b:b + 1, :])
                rand_rows.append(rrow)
            ones128 = const.tile([1, P], f32)
            nc.vector.memset(ones128[:], 1.0)
            one1 = const.tile([1, 1], f32)
            nc.vector.memset(one1[:], 1.0)
            hT1 = const.tile([HID + 1, BP], f32)
            nc.vector.memset(hT1[HID:HID + 1, :], 1.0)

            for rep in range(reps):
                # per-rep scratch (bufs=2 pools -> reps can pipeline)
                sums = smallp.tile([P, COLS], f32, name="sums", tag="sums")
                maxs = smallp.tile([P, COLS], f32, name="maxs", tag="maxs")
                stats = smallp.tile([P, COLS], f32, name="stats", tag="stats")
                gts = smallp.tile([P, COLS], f32, name="gts", tag="gts")
                zcb = smallp.tile([P, COLS], f32, name="zcb", tag="zcb")
                y2cb = smallp.tile([P, COLS], f32, name="y2cb", tag="y2cb")
                randcb = smallp.tile([P, COLS], f32, name="randcb", tag="randcb")
                mask_sb = smallp.tile([P, COLS], f32, name="mask_sb", tag="mask")
                rb_sb = smallp.tile([P, COLS], f32, name="rb_sb", tag="rb")
                u_sb = smallp.tile([P, COLS], f32, name="u_sb", tag="u")
                v_sb = smallp.tile([P, COLS], f32, name="v_sb", tag="v")
                fmap = smallp.tile([P, COLS], f32, name="fmap", tag="fmap")

                for b in range(BP):
                    sl = slice(b * NBLK, (b + 1) * NBLK)

                    # ---- load x[b] in chunks of TB channel blocks ----
                    chunks = []
                    for g0 in range(0, NBLK, TB):
                        xt = xp.tile([P, TB, S], f32, name="xt", tag="xt")
                        nc.sync.dma_start(
                            xt[:],
                            x_d.ap()[b, g0 * P:(g0 + TB) * P, :]
                            .rearrange("(m p) s -> p m s", p=P))
                        chunks.append(xt)
                    xbs = [chunks[m // TB][:, m % TB, :] for m in range(NBLK)]

                    def store_chunks():
                        for ci, g0 in enumerate(range(0, NBLK, TB)):
                            st_eng.dma_start(
                                out_d.ap()[b, g0 * P:(g0 + TB) * P, :]
                                .rearrange("(m p) s -> p m s", p=P),
                                chunks[ci][:])

                    if mode == "dmaonly":
                        store_chunks()
                        continue

                    for m in range(NBLK):
                        col = b * NBLK + m
                        tr = trashp.tile([P, S], mybir.dt.bfloat16,
                                         name="tr", tag="tr")
                        nc.scalar.activation(tr[:], xbs[m], Act.Copy,
                                             accum_out=sums[:, col:col + 1])
                        if opts["max_engine"] == "tsmax":
                            trg = trashp.tile([P, S], mybir.dt.bfloat16,
                                              name="trg", tag="trg")
                            nc.vector.tensor_scalar(
                                trg[:], xbs[m], 1.0, None,
                                op0=Alu.mult, op1=Alu.max,
                                accum_out=maxs[:, col:col + 1])
                        else:
                            nc.vector.reduce_max(maxs[:, col:col + 1],
                                                 xbs[m], axis=Ax.X)

                    # y = sum/S + max  (column layout)
                    nc.vector.scalar_tensor_tensor(
                        stats[:, sl], sums[:, sl], 1.0 / S, maxs[:, sl],
                        op0=Alu.mult, op1=Alu.add)

                    # ---- FC: h = prelu(y @ w1 + b1) as hT column ----
                    h_ps = ps_h.tile([HID, 1], f32, name="h_ps", tag="h")
                    for k in range(NBLK):
                        ck = b * NBLK + k
                        nc.tensor.matmul(h_ps[:], w1_sb[:, k, :],
                                         stats[:, ck:ck + 1],
                                         start=(k == 0), stop=False)
                    nc.tensor.matmul(h_ps[:], b1_sb[:], one1[:],
                                     start=False, stop=True)
                    h_sb = smallp.tile([HID, 1], f32, name="h_sb", tag="h_sb")
                    nc.vector.tensor_copy(h_sb[:], h_ps[:])
                    nc.vector.scalar_tensor_tensor(
                        hT1[0:HID, b:b + 1], h_sb[:], a_val, h_sb[:],
                        op0=Alu.mult, op1=prelu_op1)

                    # ---- z = hT1 @ [w2; b2]  -> logits row [1, C] ----
                    z_ps = ps_z.tile([1, C], f32, name="z_ps", tag="z")
                    nc.tensor.matmul(z_ps[:], hT1[:, b:b + 1], wb2_sb[:],
                                     start=True, stop=True)
                    z_sb = rowp.tile([1, C], f32, name="z_sb", tag="z_sb")
                    nc.vector.tensor_copy(z_sb[:], z_ps[:])

                    # broadcast logits row to all 128 partitions (bit-exact)
                    zb_ps = ps_zb.tile([P, C], f32, name="zb_ps", tag="zb")
                    nc.tensor.matmul(zb_ps[:], ones128[:], z_sb[:],
                                     start=True, stop=True)
                    zb_sb = bcp.tile([P, C], f32, name="zb_sb", tag="zb_sb")
                    nc.vector.tensor_copy(zb_sb[:], zb_ps[:])

                    # transpose logits row -> column layout (bit-exact)
                    t_ps = ps_t.tile([P, NBLK], f32, name="t_ps", tag="t")
                    for m in range(NBLK):
                        nc.tensor.matmul(t_ps[:, m:m + 1],
                                         z_sb[:, m * P:(m + 1) * P], one1[:],
                                         start=True, stop=True)
                    nc.vector.tensor_copy(zcb[:, sl], t_ps[:])
                    # sigmoid only on the column copy (the values we use)
                    nc.scalar.activation(y2cb[:, sl], t_ps[:], Act.Sigmoid)

                    # transpose rand row -> column layout
                    rt_ps = ps_t.tile([P, NBLK], f32, name="rt_ps", tag="t")
                    for m in range(NBLK):
                        nc.tensor.matmul(rt_ps[:, m:m + 1],
                                         rand_rows[b][:, m * P:(m + 1) * P],
                                         one1[:], start=True, stop=True)
                    nc.vector.tensor_copy(randcb[:, sl], rt_ps[:])

                    # ---- exact rank counts: gts[c] = #{c' : z[c'] > z[c]} ----
                    for m in range(NBLK):
                        col = b * NBLK + m
                        cmp_t = cmpp.tile([P, C], f32, name="cmp_t", tag="cmp")
                        nc.vector.tensor_scalar(
                            cmp_t[:], zb_sb[:], zcb[:, col:col + 1], None,
                            op0=Alu.is_gt, op1=Alu.add,
                            accum_out=gts[:, col:col + 1])

                    # ---- final map ----
                    nc.vector.tensor_scalar(mask_sb[:, sl], gts[:, sl],
                                            float(KTOP), None, op0=Alu.is_ge)
                    nc.vector.tensor_scalar(rb_sb[:, sl], randcb[:, sl],
                                            0.5, None, op0=Alu.is_lt)
                    nc.vector.tensor_tensor(u_sb[:, sl], rb_sb[:, sl],
                                            mask_sb[:, sl], op=Alu.mult)
                    nc.vector.scalar_tensor_tensor(
                        v_sb[:, sl], rb_sb[:, sl], -1.0, u_sb[:, sl],
                        op0=Alu.mult, op1=Alu.add)
                    nc.vector.scalar_tensor_tensor(
                        fmap[:, sl], v_sb[:, sl], 1.0, y2cb[:, sl],
                        op0=Alu.add, op1=Alu.mult)

                    # ---- scale tiles in place and store ----
                    for m in range(NBLK):
                        col = b * NBLK + m
                        use_act = (opts["mul_engine"] == "act"
                                   or (opts["mul_engine"] == "split"
                                       and m % 2 == 1)
                                   or (opts["mul_engine"] == "split31"
                                       and m != 0))
                        if use_act:
                            nc.scalar.activation(xbs[m], xbs[m], Act.Copy,
                                                 scale=fmap[:, col:col + 1])
                        else:
                            nc.vector.tensor_scalar(xbs[m], xbs[m],
                                                    fmap[:, col:col + 1], None,
                                                    op0=Alu.mult)
                    if mode == "nostore":
                        continue
                    store_chunks()

    nc.compile()
    return nc


# =========================================================================
# shared driver machinery
# =========================================================================

_BUILDERS = {"v1": _build, "v2": _build_v2}
DEFAULT_BUILDER = "v2"

_cache: dict = {}


def _get_nc(a_val: float, reps: int = 1, builder: str | None = None, **over):
    builder = builder or DEFAULT_BUILDER
    key = (builder, float(np.float32(a_val)), reps,
           tuple(sorted(over.items())))
    if key not in _cache:
        _cache[key] = _BUILDERS[builder](float(np.float32(a_val)), reps,
                                         **over)
    return _cache[key]


def _shard(inputs):
    x = np.ascontiguousarray(np.asarray(inputs["x"], dtype=np.float32))
    rand = np.ascontiguousarray(np.asarray(inputs["rand"], dtype=np.float32))
    w1 = np.ascontiguousarray(np.asarray(inputs["w1"], dtype=np.float32))
    b1 = np.ascontiguousarray(
        np.asarray(inputs["b1"], dtype=np.float32).reshape(1, HID))
    w2 = np.ascontiguousarray(np.asarray(inputs["w2"], dtype=np.float32))
    b2 = np.ascontiguousarray(
        np.asarray(inputs["b2"], dtype=np.float32).reshape(1, C))
    xr = x.reshape(NCORES, BP, C, S)
    rr = rand.reshape(NCORES, BP, C)
    in_maps = []
    for i in range(NCORES):
        in_maps.append({
            "x": np.ascontiguousarray(xr[i]),
            "rand": np.ascontiguousarray(rr[i]),
            "w1": w1, "b1": b1, "w2": w2, "b2": b2,
        })
    return in_maps


def run_sharded(inputs, trace=False, trace_cores=None, reps=1,
                builder=None, **over):
    """Run on all 8 cores; returns (full_output, BassKernelResults)."""
    nc = _get_nc(float(np.asarray(inputs["prelu_a"])), reps,
                 builder=builder, **over)
    in_maps = _shard(inputs)
    res = run_bass_kernel_spmd(nc, in_maps, core_ids=list(range(NCORES)),
                               trace=trace, trace_cores=trace_cores)
    out = np.concatenate(
        [np.asarray(r["out"], dtype=np.float32) for r in res.results], axis=0)
    return out.reshape(B, C, H, W), res


def kernel(**inputs) -> np.ndarray:
    out, _ = run_sharded(inputs, trace=False)
    return out


# ---------------------------------------------------------------------------
# benchmarking machinery (test-only; grading path is kernel() above)
# ---------------------------------------------------------------------------

class _JitRunner:
    """Cached jitted shard_map executable over 8 cores with device-resident
    inputs, mirroring bass2jax.run_bass_via_pjrt's multi-core path but
    reusable across calls (no per-call retrace / host->device transfer)."""

    def __init__(self, nc, in_maps):
        import jax
        from jax.sharding import Mesh, PartitionSpec
        from jax.experimental.shard_map import shard_map
        import concourse.mybir as mb
        from concourse import bass2jax as b2j

        b2j.install_neuronx_cc_hook()
        partition_name = (nc.partition_id_tensor.name
                          if nc.partition_id_tensor else None)
        in_names, out_names, out_avals, zero_outs = [], [], [], []
        for alloc in nc.m.functions[0].allocations:
            if not isinstance(alloc, mb.MemoryLocationSet):
                continue
            name = alloc.memorylocations[0].name
            if alloc.kind == "ExternalInput":
                if name != partition_name:
                    in_names.append(name)
            elif alloc.kind == "ExternalOutput":
                out_names.append(name)
                shape = tuple(alloc.tensor_shape)
                dtype = mb.dt.np(alloc.dtype)
                out_avals.append(jax.core.ShapedArray(shape, dtype))
                zero_outs.append(np.zeros(shape, dtype))
        n_params = len(in_names)
        all_names = in_names + out_names
        if partition_name is not None:
            all_names = all_names + [partition_name]
        self.out_names = out_names

        def _body(*args):
            operands = list(args)
            if partition_name is not None:
                operands.append(b2j.partition_id_tensor())
            outs = b2j._bass_exec_p.bind(
                *operands,
                out_avals=tuple(out_avals),
                in_names=tuple(all_names),
                out_names=tuple(out_names),
                lowering_input_output_aliases=(),
                sim_require_finite=True,
                sim_require_nnan=True,
                nc=nc,
            )
            return tuple(outs)

        devices = jax.devices()[:NCORES]
        mesh = Mesh(np.asarray(devices), ("core",))
        n_outs = len(out_names)
        in_specs = (PartitionSpec("core"),) * (n_params + n_outs)
        out_specs = (PartitionSpec("core"),) * n_outs
        self.fn = jax.jit(
            shard_map(_body, mesh=mesh, in_specs=in_specs,
                      out_specs=out_specs, check_rep=False),
            keep_unused=True,
        )
        concat_in = [
            np.concatenate([np.asarray(m[nm]) for m in in_maps], axis=0)
            for nm in in_names
        ]
        concat_zeros = [
            np.zeros((NCORES * z.shape[0], *z.shape[1:]), z.dtype)
            for z in zero_outs
        ]
        self.args = [jax.device_put(a) for a in concat_in + concat_zeros]
        jax.block_until_ready(self.args)

    def __call__(self):
        import jax
        out = self.fn(*self.args)
        jax.block_until_ready(out)
        return out


_runners: dict = {}


def _get_runner(inputs, reps, builder=None, **over):
    key = ("runner", builder or DEFAULT_BUILDER,
           float(np.asarray(inputs["prelu_a"])), reps,
           tuple(sorted(over.items())))
    if key not in _runners:
        nc = _get_nc(float(np.asarray(inputs["prelu_a"])), reps,
                     builder=builder, **over)
        _runners[key] = _JitRunner(nc, _shard(inputs))
    return _runners[key]


def bench(inputs, k_lo=2, k_hi=34, calls=80, builder=None, **over):
    """Per-iteration HW time from the slope between two in-NEFF repeat
    counts. Samples are taken as adjacent (lo, hi) pairs and differenced
    pairwise so slow drift in the ~108 ms dispatch overhead cancels."""
    import time
    r_lo = _get_runner(inputs, k_lo, builder=builder, **over)
    r_hi = _get_runner(inputs, k_hi, builder=builder, **over)
    for r in (r_lo, r_hi):
        for _ in range(3):
            r()
    diffs = []
    s_lo, s_hi = [], []
    for _ in range(calls):
        t0 = time.perf_counter(); r_lo(); tl = time.perf_counter() - t0
        t0 = time.perf_counter(); r_hi(); th = time.perf_counter() - t0
        s_lo.append(tl); s_hi.append(th)
        diffs.append(th - tl)
    d = np.array(diffs) / (k_hi - k_lo) * 1e9
    a_lo, a_hi = np.array(s_lo), np.array(s_hi)
    per_iter_ns = float(np.median(d))
    return per_iter_ns, {
        "min_lo_ms": a_lo.min() * 1e3, "min_hi_ms": a_hi.min() * 1e3,
        "per_iter_med_ns": per_iter_ns,
        "per_iter_p25_ns": float(np.percentile(d, 25)),
        "per_iter_p75_ns": float(np.percentile(d, 75)),
        "per_iter_minmin_ns": float((a_hi.min() - a_lo.min())
                                    / (k_hi - k_lo) * 1e9),
    }
